# revision 59
# baseline (speedup 1.0000x reference)
"""DiscoNetFusion Trainium2 kernel (8 NeuronCores, SPMD).

Strategy
--------
Only ego agent i=0 of each scene contributes to the output, so per scene b we
need the L_b = record_len[b] neighbor warps nbr[b,0,j], the 4-layer 1x1-conv
attention head on z=[nbr;ego], a softmax over j, and the weighted feature sum
followed by a channel MLP.

Core k handles output rows [10k, 10k+10) of ALL scenes (8 cores x 10 rows =
80 rows).  Per core there are sum(record_len)=9 (scene, agent) units; each
unit is 1600 output pixels (padded to 1664 = 13 tiles of 128).

Ego agents (j=0 of each scene) have an exact-identity warp, so the host
ships their features directly in both channel-major and pixel-major layout
and they skip the gather/lerp/transpose path entirely.

The remaining agents each run the warp path: DMA gathers from a dup-row
pixel-major fp16 source (one index fetches the 2x2 tap patch), a 7-op DVE
lerp with per-pixel weights, and a PE transpose per px tile back to
channel-major.  conv1 is split into a nbr-half and an ego-half matmul
accumulating in PSUM (the z=[nbr;ego] concat never materializes; the ego
half reuses the shared per-scene channel-major ego tile).  Conv evacuations
are balanced across the Act and DVE engines to keep both busy.

conv3 uses a block-diagonal stationary (1 matmul per piece per group of 3
agents); conv4 is FUSED into the s-transpose: per px tile a tiny matmul with
the hs3 tile as stationary and a block-column w4 moving operand writes
s[px, col] directly in pixel-major PSUM.  Softmax + attention then run in
pixel-major where every op is [128, 13, 9]-sized (~100-500ns).  The weighted
sum reuses the pixel-major nbr tiles, folds per scene, and is transposed
back by PE; the MLP bias rides as a 65th weight row against a ones row.
"""

import dataclasses
import os

import numpy as np

import concourse.bacc as bacc
import concourse.mybir as mybir
from concourse.bass_utils import run_bass_kernel_spmd
from concourse.tile import TileContext

F32 = mybir.dt.float32
F16 = mybir.dt.float16
I16 = mybir.dt.int16
Alu = mybir.AluOpType
Act = mybir.ActivationFunctionType

C = 64
H = 80
W = 160
B = 3
L = 4
EPS = 1e-5
NCORES = 8
R = H // NCORES            # output rows per core
PX = R * W                 # 1600 real pixels
NT = 13                    # px tiles of 128
PXP = NT * 128             # 1664 padded pixels
NENT = H * W               # gather source entries per agent
NIDX = 2 * PXP // 16       # idx columns per pair (16-wrapped)
HCHUNKS = [(0, 832, [(0, 512), (512, 320)]), (832, 832, [(0, 512), (512, 320)])]
# u transposes write 128-wide blocks; chunks must be tile-aligned
UCHUNKS = [(0, 896), (896, 768)]
MCHUNKS = [(0, 832, [(0, 512), (512, 320)]), (832, 768, [(0, 512), (512, 256)])]


def _wrap_idx(idx_flat):
    """[N] -> [128, N//16] wrapped-in-16-partitions, replicated to 8 groups."""
    n = idx_flat.shape[0]
    w = idx_flat.reshape(n // 16, 16).T  # [16, N//16]
    return np.tile(w, (8, 1)).astype(np.int16)


def _host_warp_prep(theta, h0):
    """Per-(unit) gather indices + lerp scalars for output rows [h0,h0+R)."""
    ys = np.linspace(-1.0, 1.0, H, dtype=np.float32)[h0 : h0 + R]
    xs = np.linspace(-1.0, 1.0, W, dtype=np.float32)
    gx, gy = np.meshgrid(xs, ys)  # [R, W]
    sx = theta[0, 0] * gx + theta[0, 1] * gy + theta[0, 2]
    sy = theta[1, 0] * gx + theta[1, 1] * gy + theta[1, 2]
    px = (sx + 1.0) * (W - 1) / 2.0
    py = (sy + 1.0) * (H - 1) / 2.0
    x0 = np.floor(px).astype(np.int64)
    y0 = np.floor(py).astype(np.int64)
    fx = (px - x0).astype(np.float32)
    fy = (py - y0).astype(np.float32)

    scale = np.ones_like(fx)
    # x handling
    x0c = np.clip(x0, 0, W - 1)
    fxp = fx.copy()
    m = x0 == W - 1          # x1 out of bounds -> drop B/D taps
    fxp[m] = 0.0
    scale[m] *= 1.0 - fx[m]
    m = x0 == -1             # x0 out of bounds -> entry at x=0 is the B tap
    x0c[m] = 0
    fxp[m] = 0.0
    scale[m] *= fx[m]
    m = (x0 < -1) | (x0 > W - 1)
    x0c[m] = 0
    fxp[m] = 0.0
    scale[m] = 0.0
    # y handling (entry [y0] holds rows y0,y0+1; row 80 half is zeros)
    y0c = np.clip(y0, 0, H - 1)
    fyp = fy.copy()
    m = y0 == -1             # row0 is the F tap
    y0c[m] = 0
    fyp[m] = 0.0
    scale[m] *= fy[m]
    m = (y0 < -1) | (y0 > H - 1)
    y0c[m] = 0
    fyp[m] = 0.0
    scale[m] = 0.0

    idx = (y0c * W + x0c).reshape(-1)
    c0 = (scale * (1.0 - fyp)).reshape(-1)
    c1 = (scale * fyp).reshape(-1)
    fxp = fxp.reshape(-1)

    pad = PXP - PX
    idx = np.concatenate([idx, np.zeros(pad, np.int64)])
    fxp = np.concatenate([fxp, np.zeros(pad, np.float32)])
    c0 = np.concatenate([c0, np.zeros(pad, np.float32)])
    c1 = np.concatenate([c1, np.zeros(pad, np.float32)])
    return idx, fxp, c0, c1


def _host_warp_mask(mask_bj, theta, h0):
    """Bilinear warp of one [H,W] mask (zero padding) for rows [h0,h0+R)."""
    ys = np.linspace(-1.0, 1.0, H, dtype=np.float32)[h0 : h0 + R]
    xs = np.linspace(-1.0, 1.0, W, dtype=np.float32)
    gx, gy = np.meshgrid(xs, ys)
    sx = theta[0, 0] * gx + theta[0, 1] * gy + theta[0, 2]
    sy = theta[1, 0] * gx + theta[1, 1] * gy + theta[1, 2]
    px = (sx + 1.0) * (W - 1) / 2.0
    py = (sy + 1.0) * (H - 1) / 2.0
    x0 = np.floor(px).astype(np.int64)
    y0 = np.floor(py).astype(np.int64)
    wx = (px - x0).astype(np.float32)
    wy = (py - y0).astype(np.float32)

    def gat(xi, yi):
        inb = ((xi >= 0) & (xi < W) & (yi >= 0) & (yi < H)).astype(np.float32)
        v = mask_bj[np.clip(yi, 0, H - 1), np.clip(xi, 0, W - 1)]
        return v * inb

    out = (
        gat(x0, y0) * (1 - wx) * (1 - wy)
        + gat(x0 + 1, y0) * wx * (1 - wy)
        + gat(x0, y0 + 1) * (1 - wx) * wy
        + gat(x0 + 1, y0 + 1) * wx * wy
    )
    return out.reshape(-1)  # [PX]


def _layout(scene_of):
    """Scene starts/counts, ego set, non-ego pairs, conv groups, col perm."""
    nb = max(scene_of) + 1
    start = [None] * nb
    cnt = [0] * nb
    for a, b in enumerate(scene_of):
        if start[b] is None:
            start[b] = a
        cnt[b] += 1
    egos = [start[b] for b in range(nb)]
    non_ego = [j for j in range(len(scene_of)) if j not in egos]
    psz = int(os.environ.get("KERNEL_PAIRSZ", "1"))
    pairs = [tuple(non_ego[i : i + psz]) for i in range(0, len(non_ego), psz)]
    order = egos + non_ego
    groups = [order[i : i + 3] for i in range(0, len(order), 3)]
    col_of = {j: i for i, j in enumerate(order)}
    return start, cnt, egos, pairs, groups, col_of


def _runs(cols):
    """Split a sorted int list into (start, len) runs of consecutive ints."""
    runs = []
    for c in cols:
        if runs and c == runs[-1][0] + runs[-1][1]:
            runs[-1] = (runs[-1][0], runs[-1][1] + 1)
        else:
            runs.append((c, 1))
    return runs


def _ap(v, offset, dims):
    """Replace the free dims of AP v (keeping partition dim)."""
    return dataclasses.replace(
        v, offset=v.offset + offset, ap=[list(v.ap[0])] + [list(d) for d in dims])


def _build_program(nagents, scene_of, src_names):
    """Build the SPMD Bass program (identical for all cores)."""
    nc = bacc.Bacc("TRN2", target_bir_lowering=False, num_devices=NCORES,
                   dynamic_dma_scratch_size=16384)
    NA = nagents
    sstart, scnt, egos, pairs, groups, col_of = _layout(scene_of)
    npairs = len(pairs)

    psrc = [
        nc.dram_tensor(nm, [2 * (NENT + 1), 2 * C], F16, kind="ExternalInput")
        for nm in src_names
    ]
    idx_all = nc.dram_tensor("idx_all", [128, npairs * NIDX], I16,
                             kind="ExternalInput")
    scal_all = nc.dram_tensor("scal_all", [128, npairs * NIDX], F16,
                              kind="ExternalInput")
    ego_all = nc.dram_tensor("ego_all", [C, B * PXP], F16, kind="ExternalInput")
    ego_pmd = nc.dram_tensor("ego_pm", [128, B * NT * C], F16,
                             kind="ExternalInput")
    cmb = nc.dram_tensor("cmb", [128, NT * 2 * NA], F16, kind="ExternalInput")
    w1t1 = nc.dram_tensor("w1t1", [2 * C, 2 * C], F16, kind="ExternalInput")
    w1t2 = nc.dram_tensor("w1t2", [C, 2 * C], F16, kind="ExternalInput")
    w1t3 = nc.dram_tensor("w1t3", [C, 2 * C], F16, kind="ExternalInput")
    w2 = nc.dram_tensor("w2", [2 * C, 32], F16, kind="ExternalInput")
    bd3 = nc.dram_tensor("bd3", [96, 96], F16, kind="ExternalInput")
    bd4 = nc.dram_tensor("bd4", [96, 3], F16, kind="ExternalInput")
    mlpw65 = nc.dram_tensor("mlpw65", [C + 1, C], F16, kind="ExternalInput")
    sb = nc.dram_tensor("sb", [128, 6], F32, kind="ExternalInput")
    cb4v = nc.dram_tensor("cb4v", [128, 1], F32, kind="ExternalInput")
    sb2 = nc.dram_tensor("sb2", [96, 1], F32, kind="ExternalInput")
    sb3 = nc.dram_tensor("sb3", [96, 1], F32, kind="ExternalInput")
    ident = nc.dram_tensor("ident", [128, 128], F16, kind="ExternalInput")
    out = nc.dram_tensor("out", [B * C, PX], F32, kind="ExternalOutput")

    with TileContext(nc) as tc:
        with (
            tc.tile_pool(name="const", bufs=1) as cpool,
            tc.tile_pool(name="zs", bufs=1) as zpool,
            tc.tile_pool(name="work", bufs=2) as wpool,
            tc.tile_pool(name="att", bufs=1) as apool,
            tc.tile_pool(name="pmm", bufs=1, space="PSUM") as pmm,
            tc.tile_pool(name="ptr", bufs=2, space="PSUM") as ptr,
        ):
            # ---- constants ----
            idx_t = cpool.tile([128, npairs * NIDX], I16)
            nc.sync.dma_start(idx_t[:], idx_all[:, :])
            scal_t = cpool.tile([128, npairs * NIDX], F16)
            nc.sync.dma_start(scal_t[:], scal_all[:, :])
            ego_t = cpool.tile([C, B * PXP], F16)
            nc.sync.dma_start(ego_t[:], ego_all[:, :])
            ego_pm = cpool.tile([128, B * NT, C], F16)
            cmb_t = cpool.tile([128, NT, 2 * NA], F16)
            t1w = cpool.tile([2 * C, 2 * C], F16)
            nc.sync.dma_start(t1w[:], w1t1[:, :])
            t2w = cpool.tile([C, 2 * C], F16)
            nc.sync.dma_start(t2w[:], w1t2[:, :])
            t3w = cpool.tile([C, 2 * C], F16)
            nc.sync.dma_start(t3w[:], w1t3[:, :])
            w2_t = cpool.tile([2 * C, 32], F16)
            nc.sync.dma_start(w2_t[:], w2[:, :])
            bd3_t = cpool.tile([96, 96], F16)
            nc.sync.dma_start(bd3_t[:], bd3[:, :])
            bd4_t = cpool.tile([96, 3], F16)
            nc.sync.dma_start(bd4_t[:], bd4[:, :])
            mlpw_t = cpool.tile([C + 1, C], F16)
            nc.sync.dma_start(mlpw_t[:], mlpw65[:, :])
            sb_t = cpool.tile([128, 6], F32)
            nc.sync.dma_start(sb_t[:], sb[:, :])
            cb4_t = cpool.tile([128, 1], F32)
            nc.sync.dma_start(cb4_t[:], cb4v[:, :])
            sb2_t = cpool.tile([96, 1], F32)
            nc.sync.dma_start(sb2_t[:], sb2[:, :])
            sb3_t = cpool.tile([96, 1], F32)
            nc.sync.dma_start(sb3_t[:], sb3[:, :])
            id_t = cpool.tile([128, 128], F16)
            nc.sync.dma_start(id_t[:], ident[:, :])

            # channel-major pair z tiles (rows = a*64+c), px-major nbr tiles
            zp_all = [zpool.tile([128, PXP], F16, name=f"zp{p}", tag=f"zp{p}")
                      for p in range(npairs)]
            nbrp_all = [zpool.tile([128, 2 * NT, C], F16, name=f"nbp{p}",
                                   tag=f"nbp{p}")
                        for p in range(npairs)]
            h1_all = {}
            # late agents'/groups' conv evacs go to DVE (its lerp work has
            # drained by then, while Act is still saturated)
            ne_flat = [j for pr in pairs for j in pr]
            dve_evac = set(ne_flat[len(ne_flat) // 2 :])
            dve_evac_g = set(range(1, len(groups)))
            # s (pixel-major) accumulates from the fused conv4+transpose mms
            s_ps = pmm.tile([128, NT, 16], F32, tag="s_ps", bufs=1)

            def evac_relu(dst, psrc_ap, bias_ap, on_dve):
                if on_dve:
                    nc.vector.tensor_scalar(dst, psrc_ap, bias_ap, 0.0,
                                            Alu.add, Alu.max)
                else:
                    nc.scalar.activation(dst, psrc_ap, Act.Relu,
                                         bias=bias_ap, scale=1.0)

            def conv1_ego(j):
                b = scene_of[j]
                h1_j = wpool.tile([128, PXP], F16, name=f"h1_{j}",
                                  tag=f"h1_{j}", bufs=1)
                h1_all[j] = h1_j
                for (o, n, mms) in HCHUNKS:
                    p1 = pmm.tile([128, 832], F32, tag="p34", bufs=2)
                    for (mo, mn) in mms:
                        nc.tensor.matmul(
                            p1[:, mo : mo + mn], t3w[:],
                            ego_t[:, b * PXP + o + mo : b * PXP + o + mo + mn],
                            start=True, stop=True)
                    evac_relu(h1_j[:, o : o + n], p1[:, 0:n], sb_t[:, 1:2],
                              j in dve_evac)

            def conv1_pair(j, p, a):
                b = scene_of[j]
                h1_j = wpool.tile([128, PXP], F16, name=f"h1_{j}",
                                  tag=f"h1_{j}", bufs=1)
                h1_all[j] = h1_j
                zp = zp_all[p]
                for (o, n, mms) in HCHUNKS:
                    p1 = pmm.tile([128, 832], F32, tag="p34", bufs=2)
                    for (mo, mn) in mms:
                        nc.tensor.matmul(
                            p1[:, mo : mo + mn],
                            t1w[C * a : C * a + C, :],
                            zp[C * a : C * a + C, o + mo : o + mo + mn],
                            start=True, stop=False)
                        nc.tensor.matmul(
                            p1[:, mo : mo + mn], t2w[:],
                            ego_t[:, b * PXP + o + mo : b * PXP + o + mo + mn],
                            start=False, stop=True)
                    evac_relu(h1_j[:, o : o + n], p1[:, 0:n], sb_t[:, 1:2],
                              j in dve_evac)

            def conv234(g):
                grp = groups[g]
                ng = len(grp)
                hs2 = wpool.tile([96, PXP], F16, tag="hs2", bufs=1)
                hs3 = wpool.tile([96, PXP], F16, tag="hs3", bufs=1)
                for (o, n, mms) in HCHUNKS:
                    sl = slice(o, o + n)
                    ph2 = pmm.tile([96, 832], F32, tag="p34", bufs=2)
                    for q, jj in enumerate(grp):
                        for (mo, mn) in mms:
                            nc.tensor.matmul(
                                ph2[32 * q : 32 * q + 32, mo : mo + mn],
                                w2_t[:],
                                h1_all[jj][:, o + mo : o + mo + mn],
                                start=True, stop=True)
                    evac_relu(hs2[0 : 32 * ng, sl], ph2[0 : 32 * ng, 0:n],
                              sb2_t[0 : 32 * ng, 0:1], g in dve_evac_g)
                    p34 = pmm.tile([96, 832], F32, tag="p34", bufs=2)
                    for (mo, mn) in mms:
                        nc.tensor.matmul(
                            p34[0 : 32 * ng, mo : mo + mn],
                            bd3_t[0 : 32 * ng, 0 : 32 * ng],
                            hs2[0 : 32 * ng, o + mo : o + mo + mn],
                            start=True, stop=True)
                    evac_relu(hs3[0 : 32 * ng, sl], p34[0 : 32 * ng, 0:n],
                              sb3_t[0 : 32 * ng, 0:1], g in dve_evac_g)
                # conv4 fused with the s transpose: per px tile,
                # s_pm[px, 3g+q] = sum_c w4[c] * h3_q[32q+c, px]
                for t in range(NT):
                    nc.tensor.matmul(
                        s_ps[:, t, 3 * g : 3 * g + ng],
                        hs3[0 : 32 * ng, 128 * t : 128 * (t + 1)],
                        bd4_t[0 : 32 * ng, 0:ng],
                        start=True, stop=True)

            # ---- ego agents: direct channel-major features, conv1 early ----
            for j in egos:
                conv1_ego(j)
            conv234(0)  # group 0 = the ego agents

            # ---- non-ego pairs: gather + lerp + transpose + conv1 ----
            done_h1 = set(egos)
            done_groups = {0}
            for p, pr in enumerate(pairs):
                na2 = len(pr)  # 2, or 1 for a trailing single
                nblk = 2 * NT if na2 == 2 else NT
                # gather: blocks are (tile, agent) interleaved with indices
                # pre-offset for agent 1; split into 2 chunks to stay under
                # the SWDGE descriptor ring size
                g_t = wpool.tile([128, nblk, 4 * C], F16, tag="g", bufs=2)
                src_flat = psrc[p][:, :].rearrange("a b -> (a b)")
                src_win = dataclasses.replace(
                    src_flat, ap=[[2 * C, 2 * (NENT + 1) - 1], [1, 4 * C]]
                )
                gchunks = [(i, min(7, nblk - i)) for i in range(0, nblk, 7)]
                for (b0, bn) in gchunks:
                    nc.gpsimd.dma_gather(
                        g_t[:, b0 : b0 + bn, :],
                        src_win,
                        idx_t[:, p * NIDX + b0 * 8 :
                              p * NIDX + (b0 + bn) * 8],
                        num_idxs=bn * 128,
                        num_idxs_reg=bn * 128,
                        elem_size=4 * C,
                        elem_step=2 * C,
                        single_packet=False,
                    )
                # ---- bilinear combine: nbr = w00*A+w10*C + w01*B+w11*D ----
                t1_t = wpool.tile([128, nblk, 2 * C], F16, tag="t1", bufs=2)
                t2_t = wpool.tile([128, nblk, 2 * C], F16, tag="t2", bufs=2)
                nbr_t = nbrp_all[p]
                wq = scal_t[:, p * NIDX : (p + 1) * NIDX]
                # lead agents split the lerp per gather chunk so the first
                # half starts as soon as chunk 1 of the gather has landed
                halves = gchunks if p < 3 else [(0, nblk)]
                for (b0, bn) in halves:
                    bs = slice(b0, b0 + bn)
                    for q, dst in ((0, t1_t[:, bs, 0:C]),
                                   (1, t1_t[:, bs, C : 2 * C]),
                                   (2, t2_t[:, bs, 0:C]),
                                   (3, t2_t[:, bs, C : 2 * C])):
                        w_ap = dataclasses.replace(
                            wq, offset=wq.offset + 8 * b0 + 2 * q,
                            ap=[list(wq.ap[0]), [8, bn], [0, C // 2], [1, 2]])
                        src = g_t[:, bs, q * C : (q + 1) * C]
                        nc.vector.tensor_tensor(
                            dst.rearrange("p a (c d) -> p a c d", d=2),
                            src.rearrange("p a (c d) -> p a c d", d=2),
                            w_ap, Alu.mult)
                    eng2 = nc.gpsimd if p >= 4 else nc.vector
                    eng2.tensor_tensor(t1_t[:, bs, 0:C], t1_t[:, bs, 0:C],
                                       t2_t[:, bs, 0:C], Alu.add)
                    nc.vector.tensor_tensor(t1_t[:, bs, C : 2 * C],
                                            t1_t[:, bs, C : 2 * C],
                                            t2_t[:, bs, C : 2 * C], Alu.add)
                    eng = nc.gpsimd if p >= 3 else nc.vector
                    eng.tensor_tensor(
                        nbr_t[:, bs, :], t1_t[:, bs, 0:C],
                        t1_t[:, bs, C : 2 * C], Alu.add)
                # ---- transpose px-major -> channel-major into zpair ----
                # each px tile transposes BOTH agents' channels at once
                zp = zp_all[p]
                nv = nbr_t[:]
                for t0 in range(0, NT, 4):
                    tn = min(4, NT - t0)
                    tr_ps = ptr.tile([128, 512], F16, tag="tr")
                    for t in range(t0, t0 + tn):
                        if na2 == 2:
                            src_t = _ap(nv, 2 * t * C, [[1, 2 * C]])
                        else:
                            src_t = _ap(nv, t * C, [[1, C]])
                        nc.tensor.transpose(
                            tr_ps[0 : 64 * na2,
                                  128 * (t - t0) : 128 * (t - t0 + 1)],
                            src_t, id_t[:])
                    nc.scalar.activation(
                        zp[0 : 64 * na2, 128 * t0 : 128 * (t0 + tn)],
                        tr_ps[0 : 64 * na2, 0 : 128 * tn], Act.Copy)
                # ---- conv1 for the pair's agents ----
                for a, j in enumerate(pr):
                    conv1_pair(j, p, a)
                    done_h1.add(j)
                for g in range(len(groups)):
                    if g not in done_groups and all(
                            jj in done_h1 for jj in groups[g]):
                        conv234(g)
                        done_groups.add(g)

            # attention-only constants load late so their transfers don't
            # contend with the gather DMAs at startup
            nc.sync.dma_start(ego_pm[:], ego_pmd[:, :].rearrange(
                "p (t c) -> p t c", c=C))
            nc.sync.dma_start(cmb_t[:], cmb[:, :].rearrange(
                "p (t a) -> p t a", a=2 * NA))
            # ---- attention in pixel-major ----
            # e = exp(relu(s_raw + cb4)) = max(exp(s_raw + cb4), 1)
            e_t = apool.tile([128, NT, NA], F16)
            nc.scalar.activation(e_t[:], s_ps[:, :, 0:NA], Act.Exp,
                                 bias=cb4_t[:, 0:1], scale=1.0)
            nc.vector.tensor_scalar_max(e_t[:], e_t[:], 1.0)
            # ep = e * (cm != 0); al = e * cm   (cm columns are s-col order)
            ep_t = apool.tile([128, NT, NA], F16)
            nc.vector.tensor_tensor(ep_t[:], e_t[:], cmb_t[:, :, NA : 2 * NA],
                                    Alu.mult)
            al_t = apool.tile([128, NT, NA], F16)
            nc.vector.tensor_tensor(al_t[:], e_t[:], cmb_t[:, :, 0:NA],
                                    Alu.mult)
            # den per scene (chain adds over the scene's agent columns)
            den_t = apool.tile([128, NT, B], F16)
            for b in range(B):
                cols = sorted(col_of[j]
                              for j in range(sstart[b], sstart[b] + scnt[b]))
                nc.vector.tensor_tensor(
                    den_t[:, :, b : b + 1], ep_t[:, :, cols[0] : cols[0] + 1],
                    ep_t[:, :, cols[1] : cols[1] + 1], Alu.add)
                for ck in cols[2:]:
                    nc.vector.tensor_tensor(
                        den_t[:, :, b : b + 1], den_t[:, :, b : b + 1],
                        ep_t[:, :, ck : ck + 1], Alu.add)
            rec_t = apool.tile([128, NT, B], F16)
            with nc.allow_low_precision(reason="den>=1, fp16 rec ok"):
                nc.vector.reciprocal(rec_t[:], den_t[:])
            # alpha = al * rec[scene], written DUPLICATED (cols 2a, 2a+1)
            # so the scale mults' broadcast AP ends in a packed [1, 2] dim
            # and keeps the DVE 2x mode (a stride-0 last dim would drop it)
            alp_t = apool.tile([128, NT, 2 * NA], F16)
            pair_of = {pr[0]: p for p, pr in enumerate(pairs)}
            scl_ego = [apool.tile([128, NT, C], F16, name=f"sce{b}")
                       for b in range(B)]
            scl_pr = [apool.tile([128, 2 * NT, C], F16, name=f"scp{p}")
                      for p in range(npairs)]
            u_pm = [apool.tile([128, NT, C], F16, name=f"upm{b}")
                    for b in range(B)]
            # per scene: alpha -> scale -> fold, so scene 0's u/MLP/output
            # chain starts while later scenes' scale mults still run
            for b in range(B):
                cols = sorted(col_of[j]
                              for j in range(sstart[b], sstart[b] + scnt[b]))
                for (c0, nj) in _runs(cols):
                    r_ap = _ap(rec_t[:], b, [[B, NT], [0, nj], [0, 2]])
                    a_src = _ap(al_t[:], c0, [[NA, NT], [1, nj], [0, 2]])
                    a_dst = _ap(alp_t[:], 2 * c0,
                                [[2 * NA, NT], [2, nj], [1, 2]])
                    nc.vector.tensor_tensor(a_dst, a_src, r_ap, Alu.mult)
                slices = {}
                for j in range(sstart[b], sstart[b] + scnt[b]):
                    a_ap = _ap(alp_t[:], 2 * col_of[j],
                               [[2 * NA, NT], [0, C // 2], [1, 2]])
                    if j == egos[b]:
                        dst = scl_ego[b]
                        srcv = ego_pm[:, b * NT : (b + 1) * NT, :]
                        slices[j] = dst[:]
                    else:
                        p = pair_of[j]
                        dst = scl_pr[p]
                        srcv = nbrp_all[p][:, 0:NT, :]
                        slices[j] = _ap(dst[:], 0, [[C, NT], [1, C]])
                    nc.vector.tensor_tensor(
                        dst[:, 0:NT, :].rearrange("p t (c d) -> p t c d", d=2),
                        srcv.rearrange("p t (c d) -> p t c d", d=2),
                        a_ap, Alu.mult)
                js = list(range(sstart[b], sstart[b] + scnt[b]))
                nc.vector.tensor_tensor(u_pm[b][:], slices[js[0]],
                                        slices[js[1]], Alu.add)
                for jk in js[2:]:
                    nc.vector.tensor_tensor(u_pm[b][:], u_pm[b][:],
                                            slices[jk], Alu.add)
            # ---- transpose u back to channel-major, MLP, write out ----
            for b in range(B):
                u_sb = apool.tile([C + 1, PXP], F16, name=f"usb{b}")
                nc.gpsimd.memset(u_sb[C : C + 1, :], 1.0)
                for (o, n) in UCHUNKS:
                    u_ps = ptr.tile([C, 896], F16, tag="tr")
                    for t in range(o // 128, (o + n) // 128):
                        nc.tensor.transpose(
                            u_ps[:, 128 * t - o : 128 * (t + 1) - o],
                            u_pm[b][:, t, :], id_t[:])
                    nc.vector.tensor_scalar(u_sb[0:C, o : o + n],
                                            u_ps[:, 0:n], 0.0, None, Alu.add)
                for (o, n, mms) in MCHUNKS:
                    mps = pmm.tile([C, 832], F32, tag="p34", bufs=2)
                    for (mo, mn) in mms:
                        nc.tensor.matmul(mps[:, mo : mo + mn], mlpw_t[:],
                                         u_sb[:, o + mo : o + mo + mn],
                                         start=True, stop=True)
                    ob = wpool.tile([C, 832], F32, tag="ob")
                    nc.scalar.activation(ob[:, 0:n], mps[:, 0:n], Act.Copy)
                    nc.sync.dma_start(out[b * C : (b + 1) * C, o : o + n],
                                      ob[:, 0:n])

    nc.compile()
    return nc


_PROG_CACHE = {}
_LAST_RES = None


def kernel(**inputs):
    x = np.asarray(inputs["x"], np.float32)
    mask = np.asarray(inputs["mask"], np.float32)
    record_len = np.asarray(inputs["record_len"])
    ptm = np.asarray(inputs["pairwise_t_matrix"], np.float32)
    rec = [int(v) for v in record_len]
    agents = [(b, j) for b in range(B) for j in range(rec[b])]
    nagents = len(agents)
    scene_of = [b for (b, j) in agents]
    NA = nagents
    sstart, scnt, egos, pairs, groups, col_of = _layout(scene_of)
    npairs = len(pairs)

    # ---- regroup x into per-scene node features ----
    node = np.zeros((B, L, C, H, W), np.float32)
    idx0 = 0
    for b, n in enumerate(rec):
        node[b, :n] = x[idx0 : idx0 + n]
        idx0 += n

    # ---- gather sources (pairs): dup-row pixel-major fp16, concatenated ----
    def agent_src(a):
        b, j = agents[a]
        feat = node[b, j]  # [C, H, W]
        ent = np.zeros((H + 1, W, 2 * C), np.float16)
        pm = feat.transpose(1, 2, 0).astype(np.float16)  # [H, W, C]
        ent[:H, :, :C] = pm
        ent[:H - 1, :, C:] = pm[1:]
        arr = np.zeros((NENT + 1, 2 * C), np.float16)
        arr[:NENT] = ent[:H].reshape(NENT, 2 * C)
        return arr

    src_names = [f"psrc{p}" for p in range(npairs)]
    src_arrs = {}
    for p, pr in enumerate(pairs):
        arr = np.zeros((2 * (NENT + 1), 2 * C), np.float16)
        for a, j in enumerate(pr):
            arr[a * (NENT + 1) : a * (NENT + 1) + NENT + 1] = agent_src(j)
        src_arrs[src_names[p]] = arr

    # ---- per-core index/scalar/mask/ego prep ----
    per_core = []
    for k in range(NCORES):
        h0 = k * R
        idx_cols = np.zeros((128, npairs * NIDX), np.int16)
        scal_cols = np.zeros((128, npairs * NIDX), np.float16)
        cmb_arr = np.zeros((128, NT * 2 * NA), np.float16)
        ego_arr = np.zeros((C, B * PXP), np.float16)
        ego_pm_arr = np.zeros((128, B * NT * C), np.float16)
        for b in range(B):
            ego = np.zeros((C, PXP), np.float16)
            ego[:, :PX] = node[b, 0][:, h0 : h0 + R, :].reshape(C, PX)
            ego_arr[:, b * PXP : (b + 1) * PXP] = ego
            # px-major: [PXP, C] -> [NT, 128, C] -> [128, NT*C]
            epm = ego.T.reshape(NT, 128, C).transpose(1, 0, 2)
            ego_pm_arr[:, b * NT * C : (b + 1) * NT * C] = epm.reshape(
                128, NT * C)
        for p, pr in enumerate(pairs):
            na2 = len(pr)
            nblk = 2 * NT if na2 == 2 else NT
            gidx = np.zeros((nblk * 128,), np.int64)
            for a, j in enumerate(pr):
                b, jj = agents[j]
                theta = ptm[b, jj, 0]
                idx, fxp, c0, c1 = _host_warp_prep(theta, h0)
                gi = gidx.reshape(NT, na2, 128)
                gi[:, a, :] = (idx + a * (NENT + 1)).reshape(NT, 128)
                w00 = (c0 * (1.0 - fxp)).astype(np.float16)
                w10 = (c1 * (1.0 - fxp)).astype(np.float16)
                w01 = (c0 * fxp).astype(np.float16)
                w11 = (c1 * fxp).astype(np.float16)
                sc = scal_cols[:, p * NIDX : (p + 1) * NIDX]
                for t in range(NT):
                    pxs = slice(128 * t, 128 * (t + 1))
                    blk = na2 * t + a
                    for q, wv in enumerate((w00, w10, w01, w11)):
                        sc[:, 8 * blk + 2 * q] = wv[pxs]
                        sc[:, 8 * blk + 2 * q + 1] = wv[pxs]
            idx_cols[:, p * NIDX : p * NIDX + nblk * 8] = _wrap_idx(gidx)
        for a, (b, j) in enumerate(agents):
            theta = ptm[b, j, 0]
            col = col_of[a]
            wm = _host_warp_mask(mask[b, j], theta, h0)
            wmp = np.zeros(PXP, np.float32)
            wmp[:PX] = wm
            wmz = (wmp != 0).astype(np.float32)
            wmz[PX:] = 1.0  # keep den >= 1 on padded pixels
            cm_pm = wmp.reshape(NT, 128).T.astype(np.float16)   # [128, NT]
            cmz_pm = wmz.reshape(NT, 128).T.astype(np.float16)
            for t in range(NT):
                cmb_arr[:, t * 2 * NA + col] = cm_pm[:, t]
                cmb_arr[:, t * 2 * NA + NA + col] = cmz_pm[:, t]
        per_core.append((idx_cols, scal_cols, cmb_arr, ego_arr, ego_pm_arr))

    # ---- shared small tensors ----
    def gf(n):
        return np.asarray(inputs[n], np.float32)

    sb = np.zeros((128, 6), np.float32)
    sb2v = np.zeros((96, 1), np.float32)
    sb3v = np.zeros((96, 1), np.float32)
    a1 = gf("g1") / np.sqrt(gf("rv1") + EPS)
    sb[:, 1] = gf("be1") + (gf("cb1") - gf("rm1")) * a1
    a2 = gf("g2") / np.sqrt(gf("rv2") + EPS)
    b2f = gf("be2") + (gf("cb2") - gf("rm2")) * a2
    a3 = gf("g3") / np.sqrt(gf("rv3") + EPS)
    b3f = gf("be3") + (gf("cb3") - gf("rm3")) * a3
    for q in range(3):
        sb2v[32 * q : 32 * q + 32, 0] = b2f
        sb3v[32 * q : 32 * q + 8, 0] = b3f

    w1f = (gf("w1") * a1[None, :]).astype(np.float16)  # [128, 128]
    w1n = w1f[0:C]
    w1e = w1f[C : 2 * C]
    w3f = (gf("w3") * a3[None, :]).astype(np.float16)  # [32, 8]
    w4f = gf("w4").astype(np.float16)                  # [8, 1]
    bd3a = np.zeros((96, 96), np.float16)
    bd4a = np.zeros((96, 3), np.float16)
    for q in range(3):
        bd3a[32 * q : 32 * q + 32, 32 * q : 32 * q + 8] = w3f
        bd4a[32 * q : 32 * q + 8, q] = w4f[:, 0]

    mlp65 = np.zeros((C + 1, C), np.float16)
    mlp65[:C] = gf("mlp_w").astype(np.float16)
    mlp65[C] = gf("mlp_b").astype(np.float16)

    shared = {
        "w1t1": np.concatenate([w1n, w1n], axis=0),
        "w1t2": w1e,
        "w1t3": w1n + w1e,
        "w2": (gf("w2") * a2[None, :]).astype(np.float16),
        "bd3": bd3a,
        "bd4": bd4a,
        "mlpw65": mlp65,
        "sb": sb,
        "sb2": sb2v,
        "sb3": sb3v,
        "cb4v": np.full((128, 1), gf("cb4")[0], np.float32),
        "ident": np.eye(128, dtype=np.float16),
    }
    shared.update(src_arrs)

    key = (nagents, tuple(scene_of))
    if key not in _PROG_CACHE:
        _PROG_CACHE[key] = _build_program(nagents, scene_of, src_names)
    nc = _PROG_CACHE[key]

    in_maps = []
    for k in range(NCORES):
        idx_cols, scal_cols, cmb_arr, ego_arr, ego_pm_arr = per_core[k]
        m = dict(shared)
        m["idx_all"] = idx_cols
        m["scal_all"] = scal_cols
        m["cmb"] = cmb_arr
        m["ego_all"] = ego_arr
        m["ego_pm"] = ego_pm_arr
        in_maps.append(m)

    trace = bool(os.environ.get("KERNEL_TRACE"))
    res = run_bass_kernel_spmd(nc, in_maps, core_ids=list(range(NCORES)),
                               trace=trace)
    global _LAST_RES
    _LAST_RES = res

    out = np.zeros((B, C, H, W), np.float32)
    for k in range(NCORES):
        o = res.results[k]["out"]  # [B*C, PX]
        out[:, :, k * R : (k + 1) * R, :] = o.reshape(B, C, R, W)
    return out
